# revision 38
# baseline (speedup 1.0000x reference)
"""Trainium2 Bass kernel for multi-head attention (B=8, N=1024, C=768, H=12, D=64).

Sharding: pure data parallelism - one batch element per NeuronCore (8 cores).
Each core computes qkv projection, softmax attention and output projection for
its [1024, 768] slice with full (replicated) weights. No collectives.

v9 dataflow (vs 203us baseline):
  - input DMAs: critical tiles (wqk m={0,6}, x) as early flat/3D DMAs on the SP
    queue; bulk weights ride the software DGE on the otherwise-idle gpsimd
    engine so the ACT queue is free for the first exps
  - 12 warm-up matmuls ramp the PE p-state during the DMA wait
  - ST psum tiles are per q-half (he|ho side by side) so both O matmuls of a
    j-step wait only on the FIRST exp of the step
  - softmax denominator: l-rows staged to DRAM, broadcast-read into [128,512],
    single fast-approx reciprocal at partition base 0, normalize deferred past
    sweep B (and into the next pair for qt1) so the PE never waits on it
  - proj pre-accumulation split k{0,1,2}/k{3,4} rides as PE fillers in pairs
    3/4/5; final k=5 + output DMA for q-half 0 runs inside pair 5's tail;
    output is bf16 (host upcasts)
"""

import sys

sys.path.insert(0, "/opt/trn_rl_repo")

import numpy as np

B, N, C = 8, 1024, 768
H, D = 12, 64
SCALE = D ** -0.5  # 0.125
NCORES = 8
KT = C // 128      # 6 k-tiles over the C contraction
QT = N // 512      # 2 q-tiles of 512
NKT = N // 128     # 8 k-tiles over sequence for attention

_CACHED = None


def _build():
    from contextlib import ExitStack

    from concourse import bacc
    import concourse.bass as bass
    import concourse.mybir as mybir
    from concourse.tile import TileContext
    from bass_rust import add_dep_helper

    f32 = mybir.dt.float32
    bf16 = mybir.dt.bfloat16
    Exp = mybir.ActivationFunctionType.Exp
    Alu = mybir.AluOpType

    nc = bacc.Bacc("TRN2", target_bir_lowering=False, debug=False)

    xT = nc.dram_tensor("xT", [C, N], bf16, kind="ExternalInput").ap()
    wqk = nc.dram_tensor("wqk", [H, 128, KT, 128], bf16, kind="ExternalInput").ap()
    wv = nc.dram_tensor("wv", [C, C], bf16, kind="ExternalInput").ap()
    wp = nc.dram_tensor("wp", [C, C], bf16, kind="ExternalInput").ap()
    bT = nc.dram_tensor("bT", [128, KT], f32, kind="ExternalInput").ap()
    yT = nc.dram_tensor("yT", [C, N], bf16, kind="ExternalOutput").ap()
    la_dram = nc.dram_tensor("la_scratch", [2, QT * 512], f32).ap()

    with TileContext(nc) as tc, ExitStack() as ctx:
        singles = ctx.enter_context(tc.tile_pool(name="singles", bufs=1))
        pua_pool = ctx.enter_context(tc.tile_pool(name="pua", bufs=3))
        pub_pool = ctx.enter_context(tc.tile_pool(name="pub", bufs=9))
        y_pool = ctx.enter_context(tc.tile_pool(name="y", bufs=3))
        # PSUM: st (2 tiles x 2 banks) + o (2 x 1) + mm (2 x 1) = 8 banks
        mm_ps = ctx.enter_context(tc.tile_pool(name="mm_ps", bufs=2, space="PSUM"))
        o_ps = ctx.enter_context(tc.tile_pool(name="o_ps", bufs=2, space="PSUM"))
        st_ps = ctx.enter_context(tc.tile_pool(name="st_ps", bufs=2, space="PSUM"))

        # ---- persistent SBUF ----
        xT_sb = singles.tile([128, KT, N], bf16)          # 12 KB/part
        wqk_sb = singles.tile([128, H, KT, 128], bf16)    # 18 KB/part (m-major)
        wv_sb = singles.tile([128, KT, C], bf16)          # 9 KB/part
        wp_sb = singles.tile([128, KT, C], bf16)          # 9 KB/part
        bT_sb = singles.tile([128, KT], f32)
        qkT_sb = singles.tile([128, 2 * H, N], bf16)      # 24 KB/part
        v_sb = singles.tile([128, NKT, H, 65], bf16)      # 12.2 KB/part
        ouT_sb = singles.tile([128, KT, N], bf16)         # 12 KB/part
        yp_sb = singles.tile([128, KT, QT, 512], f32)     # 24 KB/part (proj partials)
        lrow_sb = singles.tile([65, QT, 512], f32)        # l rows staged out of PSUM
        rb_sb = singles.tile([128, QT, 512], f32)         # broadcast l, then 1/l
        warm_sb = singles.tile([128, 512], bf16)          # PE p-state warm-up operand

        # memsets first (DVE idle, no deps)
        nc.vector.memset(warm_sb, 0.0)
        nc.vector.memset(v_sb[:, :, :, 64:65], 1.0)

        # ---- input DMAs ----
        # critical path split across both HW DGE queues: SP gets wqk m=0 and
        # x qt0 (gates the first qk units), ACT gets x qt1 + the bulk weights
        x_r = xT.rearrange("(k p) n -> p k n", p=128)
        wv_r = wv.rearrange("(k p) c -> p k c", p=128)
        wqk_r = wqk.rearrange("h p k c -> p h k c")
        nc.sync.dma_start(out=wqk_sb[:, 0], in_=wqk[0])
        nc.sync.dma_start(out=xT_sb[:, 0:3, 0:512], in_=x_r[:, 0:3, 0:512])
        nc.sync.dma_start(out=bT_sb, in_=bT[:, :])
        nc.scalar.dma_start(out=xT_sb[:, 3:6, 0:512], in_=x_r[:, 3:6, 0:512])
        nc.scalar.dma_start(out=wqk_sb[:, H // 2], in_=wqk[H // 2])
        nc.scalar.dma_start(out=xT_sb[:, :, 512:1024], in_=x_r[:, :, 512:1024])
        nc.scalar.dma_start(out=wv_sb[:, :, 0:256], in_=wv_r[:, :, 0:256])
        nc.scalar.dma_start(out=wqk_sb[:, 1:6], in_=wqk_r[:, 1:6])
        nc.scalar.dma_start(out=wqk_sb[:, 7:12], in_=wqk_r[:, 7:12])
        nc.scalar.dma_start(out=wv_sb[:, :, 256:512], in_=wv_r[:, :, 256:512])
        nc.scalar.dma_start(out=wv_sb[:, :, 512:768], in_=wv_r[:, :, 512:768])
        wp_r = wp.rearrange("(k p) c -> p k c", p=128)
        nc.scalar.dma_start(out=wp_sb, in_=wp_r)

        # ---- PE p-state warm-up: dummy matmuls on zeroed scratch ----
        for w in range(16):
            ps = mm_ps.tile([128, 512], f32, tag="mm", name=f"warm_{w}")
            nc.tensor.matmul(ps, warm_sb[:, 0:128], warm_sb, start=True, stop=True)

        def qk_unit(t, u):
            """One (m, qt) unit of the qk-pass for head pair t (u in 0..3)."""
            m = t if u < 2 else (H // 2) + t
            qt = u % 2
            ps = mm_ps.tile([128, 512], f32, tag="mm", name=f"qk_{m}_{qt}")
            for k in range(KT):
                nc.tensor.matmul(
                    ps,
                    wqk_sb[:, m, k, :],
                    xT_sb[:, k, qt * 512:(qt + 1) * 512],
                    start=(k == 0),
                    stop=(k == KT - 1),
                )
            nc.vector.tensor_copy(qkT_sb[:, m, qt * 512:(qt + 1) * 512], ps)

        def v_chunk(j, c0, csz):
            """v[n-tile j, c0:c0+csz] = x @ w_v chunk (natural, n on partitions)."""
            ps = mm_ps.tile([128, 512], f32, tag="mm", name=f"v_{j}_{c0}")
            for k in range(KT):
                nc.tensor.matmul(
                    ps[:, 0:csz],
                    xT_sb[:, k, j * 128:(j + 1) * 128],
                    wv_sb[:, k, c0:c0 + csz],
                    start=(k == 0),
                    stop=(k == KT - 1),
                )
            nh = csz // 64
            nc.vector.tensor_copy(
                v_sb[:, j, c0 // 64:c0 // 64 + nh, 0:64],
                ps[:, 0:csz].rearrange("p (h c) -> p h c", c=64),
            )

        def proj_a(m, qt):
            """yp = sum_k(0..2) wp_k^T ouT_k + bias  (PE filler, pairs 3/4)."""
            ps = mm_ps.tile([128, 512], f32, tag="mm", name=f"ypa_{m}_{qt}")
            for k in range(3):
                nc.tensor.matmul(
                    ps,
                    wp_sb[:, k, m * 128:(m + 1) * 128],
                    ouT_sb[:, k, qt * 512:(qt + 1) * 512],
                    start=(k == 0),
                    stop=(k == 2),
                )
            nc.vector.tensor_scalar(
                yp_sb[:, m, qt], ps, bT_sb[:, m:m + 1], None, Alu.add
            )

        def proj_b(m, qt):
            """yp += sum_k(3..4) wp_k^T ouT_k  (PE filler, pair 5)."""
            ps = mm_ps.tile([128, 512], f32, tag="mm", name=f"ypb_{m}_{qt}")
            for k in (3, 4):
                nc.tensor.matmul(
                    ps,
                    wp_sb[:, k, m * 128:(m + 1) * 128],
                    ouT_sb[:, k, qt * 512:(qt + 1) * 512],
                    start=(k == 3),
                    stop=(k == 4),
                )
            nc.vector.tensor_tensor(yp_sb[:, m, qt], yp_sb[:, m, qt], ps, Alu.add)

        def proj_final_mms(qt):
            """k=5 matmuls for all 6 m-tiles, spread over all three PSUM pools
            so no eviction-paced reuse stalls the PE."""
            pss = []
            pools = ((mm_ps, "mm"), (st_ps, "st"), (o_ps, "o"))
            for m in range(KT):
                pool, tag = pools[m % 3]
                ps = pool.tile([128, 512], f32, tag=tag, name=f"y_{m}_{qt}")
                nc.tensor.matmul(
                    ps,
                    wp_sb[:, KT - 1, m * 128:(m + 1) * 128],
                    ouT_sb[:, KT - 1, qt * 512:(qt + 1) * 512],
                    start=True,
                    stop=True,
                )
                pss.append(ps)
            return pss

        def proj_final_evict(qt, pss):
            for m, ps in enumerate(pss):
                yt = y_pool.tile([128, 512], bf16, tag="y")
                nc.vector.tensor_tensor(yt, ps, yp_sb[:, m, qt], Alu.add)
                nc.sync.dma_start(
                    out=yT[m * 128:(m + 1) * 128, qt * 512:(qt + 1) * 512], in_=yt
                )

        def proj_final(qt):
            proj_final_evict(qt, proj_final_mms(qt))

        def drain_pre(t, qt, tiles, on_act=False):
            """Stage PSUM l-rows to DRAM, broadcast-read the raw l across all
            partitions. No PE involvement; normalize happens in drain_post."""
            he, ho = 2 * t, 2 * t + 1
            q0 = qt * 512

            def roundtrip(po, r, dma_eng):
                wr = dma_eng.dma_start(
                    out=la_dram[r:r + 1, q0:q0 + 512], in_=lrow_sb[po:po + 1, qt, :]
                )
                rd = dma_eng.dma_start(
                    out=rb_sb[po:po + 64, qt],
                    in_=la_dram[r:r + 1, q0:q0 + 512].to_broadcast([64, 512]),
                )
                add_dep_helper(rd.ins, wr.ins, reason="la dram write->read")

            if on_act:
                # tail: ACT engine is idle; l copies first so both DMA
                # round trips (ho on the ACT queue, he on SP) launch early
                nc.scalar.copy(lrow_sb[64:65, qt, :], tiles[ho][64:65, :])
                roundtrip(64, 1, nc.scalar)
                nc.scalar.copy(lrow_sb[0:1, qt, :], tiles[he][64:65, :])
                roundtrip(0, 0, nc.sync)
                nc.scalar.copy(ouT_sb[64:128, t, q0:q0 + 512], tiles[ho][0:64, :])
                nc.scalar.copy(ouT_sb[0:64, t, q0:q0 + 512], tiles[he][0:64, :])
            else:
                for h, po, r in ((he, 0, 0), (ho, 64, 1)):
                    nc.vector.tensor_copy(lrow_sb[po:po + 1, qt, :], tiles[h][64:65, :])
                    roundtrip(po, r, nc.sync)
                    nc.vector.tensor_copy(
                        ouT_sb[po:po + 64, t, q0:q0 + 512], tiles[h][0:64, :]
                    )
            # one fast reciprocal over the whole broadcast tile (base partition 0)
            nc.vector.reciprocal_approx_fast(rb_sb[:, qt, :], rb_sb[:, qt, :])

        def drain_post(t, qt):
            """Normalize ouT by the broadcast 1/l."""
            q0 = qt * 512
            nc.vector.tensor_tensor(
                ouT_sb[:, t, q0:q0 + 512], ouT_sb[:, t, q0:q0 + 512],
                rb_sb[:, qt], Alu.mult,
            )

        def attention_pair(t, extras_by_j=None):
            he, ho = 2 * t, 2 * t + 1
            mt_q, mt_k = t, (H // 2) + t
            pu1 = {}
            o_tiles = {
                h: o_ps.tile([65, 512], f32, tag="o", name=f"o_h{h}_q0")
                for h in (he, ho)
            }
            for j in range(NKT):
                # ST: q-half tiles, he in cols 0:512 / ho in 512:1024
                sts = {}
                for qt in range(QT):
                    st = st_ps.tile([128, 1024], f32, tag="st", name=f"st{qt}_{t}_{j}")
                    for h, po, c0 in ((he, 0, 0), (ho, 64, 512)):
                        nc.tensor.matmul(
                            st[:, c0:c0 + 512],
                            qkT_sb[po:po + 64, mt_k, j * 128:(j + 1) * 128],
                            qkT_sb[po:po + 64, mt_q, qt * 512:(qt + 1) * 512],
                            start=True,
                            stop=True,
                        )
                    sts[qt] = st
                pu0 = pua_pool.tile([128, 1024], bf16, tag="pua", name=f"pu0_{t}_{j}")
                nc.scalar.activation(pu0, sts[0], Exp, scale=SCALE)
                pu1[j] = pub_pool.tile([128, 1024], bf16, tag="pub", name=f"pu1_{t}_{j}")
                nc.scalar.activation(pu1[j], sts[1], Exp, scale=SCALE)
                if extras_by_j and (j, 0) in extras_by_j:
                    for thunk in extras_by_j[(j, 0)]:
                        thunk()
                for h, c0 in ((he, 0), (ho, 512)):
                    nc.tensor.matmul(
                        o_tiles[h],
                        v_sb[:, j, h, :],
                        pu0[:, c0:c0 + 512],
                        start=(j == 0),
                        stop=(j == NKT - 1),
                    )
                if extras_by_j and (j, 1) in extras_by_j:
                    for thunk in extras_by_j[(j, 1)]:
                        thunk()
            drain_pre(t, 0, o_tiles)
            if t == KT - 1:
                # cover the drain chain with the remaining proj partials
                for m in range(2, KT):
                    proj_b(m, 1)
            # sweep B: O for q-tile 1 (re-reads retained PuT qt1 tiles);
            # its matmuls cover the qt0 drain latency before drain_post
            o_tiles2 = {
                h: o_ps.tile([65, 512], f32, tag="o", name=f"o_h{h}_q1")
                for h in (he, ho)
            }
            for j in range(NKT):
                for h, c0 in ((he, 0), (ho, 512)):
                    nc.tensor.matmul(
                        o_tiles2[h],
                        v_sb[:, j, h, :],
                        pu1[j][:, c0:c0 + 512],
                        start=(j == 0),
                        stop=(j == NKT - 1),
                    )
            drain_post(t, 0)
            drain_pre(t, 1, o_tiles2, on_act=(t == KT - 1))
            if t == KT - 1:
                # final qt0 proj matmuls fill the PE while the qt1 drain
                # round-trips; the qt1 normalize beats their evictions to DVE
                pss0 = proj_final_mms(0)
                drain_post(t, 1)
                proj_final_evict(0, pss0)

        # ---- filler schedule: units placed into (j, phase) slots of each pair;
        # phase 0 = between exp and O (covers exp latency), phase 1 = after O ----
        def extras(t, pending=None):
            e = {}

            def put(j, ph, thunk):
                e.setdefault((j, ph), []).append(thunk)

            if pending is not None:
                put(0, 0, pending)
            if t == 0:
                # v heads 0-3 before O (pair 0 consumes h 0/1), heads 4-7 after
                for j in range(NKT):
                    put(j, 0, lambda j=j: v_chunk(j, 0, 256))
                    put(j, 1, lambda j=j: v_chunk(j, 256, 256))
                for u, j in enumerate((1, 3, 5, 7)):
                    put(j, 1, lambda u=u: qk_unit(1, u))
            elif t == 1:
                for u, j in enumerate((1, 3, 5, 7)):
                    put(j, 0, lambda u=u: qk_unit(2, u))
                for u, j in enumerate((2, 6)):
                    put(j, 0, lambda u=u: qk_unit(3, u))
            elif t == 2:
                for u, j in enumerate((1, 5)):
                    put(j, 0, lambda u=u: qk_unit(3, u + 2))
                for u, j in enumerate((3, 7)):
                    put(j, 0, lambda u=u: qk_unit(4, u))
            elif t == 3:
                for u, j in enumerate((1, 5)):
                    put(j, 0, lambda u=u: qk_unit(4, u + 2))
                for u, j in enumerate((3, 7)):
                    put(j, 0, lambda u=u: qk_unit(5, u))
                for i, j in enumerate((0, 2, 4)):
                    put(j, 0, lambda i=i: proj_a(2 * i, 0))
                    put(j, 1, lambda i=i: proj_a(2 * i + 1, 0))
            elif t == 4:
                for u, j in enumerate((1, 3)):
                    put(j, 0, lambda u=u: qk_unit(5, u + 2))
                # v heads 8-11 before O (pair 4 consumes h 8/9)
                for j in range(NKT):
                    put(j, 0, lambda j=j: v_chunk(j, 512, 256))
                for i, j in enumerate((2, 4, 6)):
                    put(j, 1, lambda i=i: proj_a(2 * i, 1))
                    put(j, 1, lambda i=i: proj_a(2 * i + 1, 1))
            elif t == 5:
                for m in range(KT):
                    j = m * 8 // KT
                    put(j, m % 2, lambda m=m: proj_b(m, 0))
                put(6, 0, lambda: proj_b(0, 1))
                put(7, 0, lambda: proj_b(1, 1))
            return e

        for u in range(4):
            qk_unit(0, u)
        for t in range(KT):
            # the qt1 normalize of the previous pair fills this pair's first slot
            pending = (lambda t=t: drain_post(t - 1, 1)) if t > 0 else None
            attention_pair(t, extras(t, pending))
        proj_final(1)

    nc.compile()
    return nc


def _get_nc():
    global _CACHED
    if _CACHED is None:
        _CACHED = _build()
    return _CACHED


def _prep_inputs(x, w_qkv, w_proj, b_proj):
    """Host-side prep: per-core input dict list (bf16, pre-transposed)."""
    import ml_dtypes

    x = np.asarray(x, dtype=np.float32)
    w_qkv = np.asarray(w_qkv, dtype=np.float32)
    w_proj = np.asarray(w_proj, dtype=np.float32)
    b_proj = np.asarray(b_proj, dtype=np.float32)

    wqk_t = np.ascontiguousarray(
        w_qkv[:, : 2 * C].astype(ml_dtypes.bfloat16)
        .reshape(KT, 128, H, 128).transpose(2, 1, 0, 3)
    )
    wv = np.ascontiguousarray(w_qkv[:, 2 * C:].astype(ml_dtypes.bfloat16))
    wp = np.ascontiguousarray(w_proj.astype(ml_dtypes.bfloat16))
    bT = np.ascontiguousarray(b_proj.reshape(KT, 128).T)

    in_maps = []
    for b in range(B):
        in_maps.append(
            {
                "xT": np.ascontiguousarray(x[b].T.astype(ml_dtypes.bfloat16)),
                "wqk": wqk_t,
                "wv": wv,
                "wp": wp,
                "bT": bT,
            }
        )
    return in_maps


def kernel(x, w_qkv, w_proj, b_proj):
    from concourse.bass_utils import run_bass_kernel_spmd

    nc = _get_nc()
    in_maps = _prep_inputs(x, w_qkv, w_proj, b_proj)
    res = run_bass_kernel_spmd(nc, in_maps, list(range(NCORES)))
    out = np.empty((B, N, C), dtype=np.float32)
    for b in range(B):
        out[b] = res.results[b]["yT"].astype(np.float32).T
    return out
